# revision 5
# baseline (speedup 1.0000x reference)
"""Trainium2 Bass kernel for the ASG adjacency problem (v3, packed-symmetric).

Computes, for batched inputs async_fea [B,N,D] and coord [B,N,2]:
    fn   = async_fea / ||async_fea||_row      (host, f64 -> bf16)
    cos  = fn @ fn^T                          (PE, bf16 x bf16 -> f32 PSUM)
    d2   = (xi-xj)^2 + (yi-yj)^2              (exact f32 in reference rounding
                                               order: sync path needs this;
                                               8 pairs sit within 1e-6 of 1.0)
    async_adj = cos * exp(-sqrt(d2))          (bf16 out; loose-tol path)
    sync_adj  = (d2 < 1.0)                    (uint8 out)
Outputs are symmetric: the device computes only the packed upper block
triangle ([P, SW] per batch, SW=1280) and the host mirrors the lower
blocks, zeroes the diagonal, and upcasts to f32.

Packed block offsets are permuted (_OFF = [0, 512, 1024, 896]) so each
block's span stays inside a single 2KB PSUM bank, letting all 4 cos
matmuls target one [P, SW] PSUM region and the asy multiply run as a
single 1280-wide DVE op.

Sharding: data-parallel over batch: 8 NeuronCores x 8 batches.

Engine split (measured: ACT 0.92 / DVE-tt 0.69 / Pool 2.2-2.4 ns per
128-lane elem-row; DVE tensor_scalar is 2.5x slower than tensor_tensor,
so all DVE ops use tensor_tensor with stride-0 [P,1] broadcasts):
 - ACT: x-squares blocks {0,1}, y-squares block {2} (Square scale=-1
   bias: exact), dist=sqrt(d2), e=exp(-dist) -- sqrt/exp pair-wide
 - DVE: x-subs blocks {2,3} + xmul, y-subs blocks {0,1,3} + ymul,
   syn=is_lt (pair-wide), asy=cos*e (one 1280-wide op per batch)
 - Pool: d2 = xsq + ysq (pair-wide, in-place into xsq)
ACT table thrash avoided by phase-grouping (Square -> Sqrt -> Exp) per
GRP=4 batches, chained with scheduling-only deps.
"""

from contextlib import ExitStack

import numpy as np

import concourse.bacc as bacc
import concourse.bass as bass
import concourse.tile as tile
from concourse import mybir
from concourse.tile_rust import add_dep_helper

P = 128          # partitions
N = 512          # nodes per batch
D = 128          # feature dim
B = 64           # total batches
NCORES = 8
BPC = B // NCORES   # batches per core
NPAIR = BPC // 2    # batch pairs per core
NB = N // P         # 4 row blocks
SW = NB * (NB + 1) // 2 * P  # packed upper-tri width: 1280
GRP = 4             # batches per ACT phase group
F32 = mybir.dt.float32
BF16 = mybir.dt.bfloat16
U8 = mybir.dt.uint8

_AF = mybir.ActivationFunctionType
_OP = mybir.AluOpType

# packed column offset of row-block i inside the [P, SW] tiles; permuted so
# every block [off, off+W) stays inside one 2KB PSUM bank
_OFF = [0, 512, 1024, 896]
_W = [512, 384, 256, 128]


def _bc(ap, w):
    """[P,1] AP -> [P,w] stride-0 free-dim broadcast."""
    return bass.AP(ap.tensor, ap.offset, [[ap.ap[0][0], P], [0, w]])


def _build_module() -> bass.Bass:
    nc = bacc.Bacc(
        "TRN2", target_bir_lowering=False, debug=False, num_devices=NCORES
    )
    # fnT2[pr, d, q*N + j] = fn[2*pr+q, j, d]  (host-normalized, bf16)
    fnT2 = nc.declare_dram_parameter("fnT2", [NPAIR, P, 2 * N], BF16, isOutput=False)
    # cpair[pr, 0, :] = [x_{b0}(N) | y_{b0}(N) | x_{b1}(N) | y_{b1}(N)]
    cpair = nc.declare_dram_parameter("cpair", [NPAIR, 1, 4 * N], F32, isOutput=False)
    # scal[p, b*8 + c*4 + k] = coord[b, k*128+p, c]
    scal = nc.declare_dram_parameter("scal", [P, BPC * 8], F32, isOutput=False)
    oasy = nc.declare_dram_parameter("oasy", [NPAIR, P, 2 * SW], BF16, isOutput=True)
    osyn = nc.declare_dram_parameter("osyn", [NPAIR, P, 2 * SW], U8, isOutput=True)

    with tile.TileContext(nc) as tc, ExitStack() as ctx:
        _build_kernel(ctx, tc, fnT2, cpair, scal, oasy, osyn)
    nc.finalize()
    return nc


def _build_kernel(ctx, tc, fnT2, cpair, scal, oasy, osyn):
    nc = tc.nc
    prev_phase = []
    cur_acts = []

    def act(*args, **kwargs):
        """ScalarE activation ordered after every op of the previous
        *phase* so activation-table-load locality holds."""
        inst = nc.scalar.activation(*args, **kwargs)
        for p in prev_phase:
            add_dep_helper(inst.ins, p.ins, False, "act phase order")
        cur_acts.append(inst)
        return inst

    def act_phase():
        if cur_acts:
            prev_phase[:] = cur_acts
            cur_acts.clear()

    const_pool = ctx.enter_context(tc.tile_pool(name="const", bufs=1))
    cb_pool = ctx.enter_context(tc.tile_pool(name="cb", bufs=3))
    fn_pool = ctx.enter_context(tc.tile_pool(name="fn", bufs=3))
    sq_pool = ctx.enter_context(tc.tile_pool(name="sq", bufs=3))
    ty_pool = ctx.enter_context(tc.tile_pool(name="ty", bufs=2))
    de_pool = ctx.enter_context(tc.tile_pool(name="de", bufs=3))
    out_pool = ctx.enter_context(tc.tile_pool(name="outp", bufs=2))
    ps_pool = ctx.enter_context(tc.tile_pool(name="ps", bufs=2, space="PSUM"))

    scal_sb = const_pool.tile([P, BPC * 8], F32)
    nc.sync.dma_start(out=scal_sb[:], in_=scal[:])
    one_c = const_pool.tile([P, 1], F32)
    nc.vector.memset(one_c[:], 1.0)

    def sx(b, k):
        return scal_sb[:, b * 8 + k : b * 8 + k + 1]

    def sy(b, k):
        return scal_sb[:, b * 8 + 4 + k : b * 8 + 4 + k + 1]

    for g in range(BPC // GRP):
        prs = range(g * (GRP // 2), (g + 1) * (GRP // 2))

        cbs = {}
        fns = {}
        for pr in prs:
            cb = cb_pool.tile([P, 4 * N], F32, name="cb")
            a = cpair[pr, 0:1, :]
            nc.sync.dma_start(
                out=cb[:],
                in_=bass.AP(a.tensor, a.offset, [[1, 1], [0, P], [1, 4 * N]]),
            )
            cbs[pr] = cb
            fnt = fn_pool.tile([P, 2 * N], BF16, name="fnt")
            nc.sync.dma_start(out=fnt[:], in_=fnT2[pr])
            fns[pr] = fnt

        # ---- Phase A (Square table): squares, d2, syn -------------------
        d2s = {}
        syn2s = {}
        for pr in prs:
            cb = cbs[pr]
            xsq2 = sq_pool.tile([P, 2 * SW], F32, name="xsq2")
            ysq2 = ty_pool.tile([P, 2 * SW], F32, name="ysq2")
            for q in (0, 1):
                b = 2 * pr + q
                xof = q * 2 * N        # x row of batch q inside cb
                yof = q * 2 * N + N    # y row of batch q inside cb
                o = q * SW             # packed offset of batch q

                for i in range(NB):
                    W, c0, off = _W[i], i * P, _OFF[i]
                    if i < 2:
                        # ACT x-squares blocks 0,1 (exact)
                        act(out=xsq2[:, o + off : o + off + W],
                            in_=cb[:, xof + c0 : xof + N],
                            func=_AF.Square, bias=sx(b, i), scale=-1.0)
                        # DVE y-subs blocks 0,1
                        nc.vector.tensor_tensor(
                            out=ysq2[:, o + off : o + off + W],
                            in0=cb[:, yof + c0 : yof + N],
                            in1=_bc(sy(b, i), W), op=_OP.subtract,
                        )
                    elif i == 2:
                        # ACT y-square block 2 (exact)
                        act(out=ysq2[:, o + off : o + off + W],
                            in_=cb[:, yof + c0 : yof + N],
                            func=_AF.Square, bias=sy(b, i), scale=-1.0)
                        # DVE x-sub block 2
                        nc.vector.tensor_tensor(
                            out=xsq2[:, o + off : o + off + W],
                            in0=cb[:, xof + c0 : xof + N],
                            in1=_bc(sx(b, i), W), op=_OP.subtract,
                        )
                    else:
                        # DVE x-sub and y-sub block 3
                        nc.vector.tensor_tensor(
                            out=xsq2[:, o + off : o + off + W],
                            in0=cb[:, xof + c0 : xof + N],
                            in1=_bc(sx(b, i), W), op=_OP.subtract,
                        )
                        nc.vector.tensor_tensor(
                            out=ysq2[:, o + off : o + off + W],
                            in0=cb[:, yof + c0 : yof + N],
                            in1=_bc(sy(b, i), W), op=_OP.subtract,
                        )
                # xmul over blocks 3,2 = [896:1280); ymul over blocks 0,1,3 = [0:1024)
                nc.vector.tensor_mul(
                    xsq2[:, o + 896 : o + SW],
                    xsq2[:, o + 896 : o + SW], xsq2[:, o + 896 : o + SW],
                )
                nc.vector.tensor_mul(
                    ysq2[:, o : o + 1024],
                    ysq2[:, o : o + 1024], ysq2[:, o : o + 1024],
                )
            # d2 = xsq + ysq in place (exact f32 add), pair-wide
            nc.gpsimd.tensor_add(xsq2[:], xsq2[:], ysq2[:])
            d2s[pr] = xsq2
            syn2 = out_pool.tile([P, 2 * SW], U8, name="syn2")
            nc.vector.tensor_tensor(
                out=syn2[:], in0=xsq2[:], in1=_bc(one_c[:, 0:1], 2 * SW),
                op=_OP.is_lt,
            )
            syn2s[pr] = syn2
            nc.sync.dma_start(out=osyn[pr], in_=syn2[:])

        act_phase()
        # ---- Phase B (Sqrt): dist ---------------------------------------
        dists = {}
        for pr in prs:
            dist2 = de_pool.tile([P, 2 * SW], BF16, name="dist2")
            act(out=dist2[:], in_=d2s[pr][:], func=_AF.Sqrt)
            dists[pr] = dist2

        act_phase()
        # ---- Phase C (Exp): e, cos, asy ---------------------------------
        for pr in prs:
            fnt = fns[pr]
            e2 = de_pool.tile([P, 2 * SW], BF16, name="e2")
            act(out=e2[:], in_=dists[pr][:], func=_AF.Exp, scale=-1.0)
            asy2 = out_pool.tile([P, 2 * SW], BF16, name="asy2")
            for q in (0, 1):
                b = 2 * pr + q
                # [P, 1536] = 3 PSUM banks so block spans stay bank-local
                ps = ps_pool.tile([P, 1536], F32, name="ps")
                for i in range(NB):
                    W, c0, off = _W[i], i * P, _OFF[i]
                    nc.tensor.matmul(
                        ps[:, off : off + W],
                        lhsT=fnt[:, q * N + c0 : q * N + c0 + P],
                        rhs=fnt[:, q * N + c0 : (q + 1) * N],
                        start=True, stop=True,
                    )
                nc.vector.tensor_mul(
                    asy2[:, q * SW : (q + 1) * SW], ps[:, :SW],
                    e2[:, q * SW : (q + 1) * SW],
                )
            nc.sync.dma_start(out=oasy[pr], in_=asy2[:])
        act_phase()


_NC_CACHE = None


def _get_module():
    global _NC_CACHE
    if _NC_CACHE is None:
        _NC_CACHE = _build_module()
    return _NC_CACHE


def _prep_inputs(async_fea: np.ndarray, coord: np.ndarray):
    import ml_dtypes

    fea = np.asarray(async_fea, dtype=np.float32)
    crd = np.asarray(coord, dtype=np.float32)
    # host-side row normalization (f64 for accuracy; cos path is loose-tol)
    nrm = np.maximum(
        np.sqrt((fea.astype(np.float64) ** 2).sum(-1, keepdims=True)), 1e-8
    )
    fn = (fea.astype(np.float64) / nrm).astype(ml_dtypes.bfloat16)

    in_maps = []
    for c in range(NCORES):
        sl = slice(c * BPC, (c + 1) * BPC)
        fn_c = fn[sl]                      # [BPC, N, D]
        # fnT2[pr, d, q*N + j] = fn[2pr+q, j, d]
        fnT2 = np.ascontiguousarray(
            fn_c.reshape(NPAIR, 2, N, D).transpose(0, 3, 1, 2).reshape(
                NPAIR, D, 2 * N
            )
        )
        cT = crd[sl].transpose(0, 2, 1)    # [BPC, 2, N]
        # [pr, q, c, N] row-major -> [x_b0 | y_b0 | x_b1 | y_b1]
        cpair = np.ascontiguousarray(cT.reshape(NPAIR, 1, 4 * N))
        # scal[p, b*8 + c*4 + k] = coord[b, k*128+p, c]
        sc = np.ascontiguousarray(
            cT.reshape(BPC, 2, NB, P).transpose(3, 0, 1, 2).reshape(P, BPC * 8)
        )
        in_maps.append({"fnT2": fnT2, "cpair": cpair, "scal": sc})
    return in_maps


def _unpack(res) -> np.ndarray:
    """Packed per-core [NPAIR, P, 2*SW] outputs -> full [2, B, N, N] f32."""
    asy = np.concatenate(
        [np.asarray(res.results[c]["oasy"]) for c in range(NCORES)], axis=0
    )
    syn = np.concatenate(
        [np.asarray(res.results[c]["osyn"]) for c in range(NCORES)], axis=0
    )
    asy = asy.reshape(B // 2, P, 2, SW).transpose(0, 2, 1, 3).reshape(B, P, SW)
    syn = syn.reshape(B // 2, P, 2, SW).transpose(0, 2, 1, 3).reshape(B, P, SW)

    out = np.empty((2, B, N, N), dtype=np.float32)
    for i in range(NB):
        W, c0, off = _W[i], i * P, _OFF[i]
        out[0, :, c0 : c0 + P, c0:] = asy[:, :, off : off + W]
        out[1, :, c0 : c0 + P, c0:] = syn[:, :, off : off + W]
    # mirror lower blocks from the (computed) upper blocks
    for i in range(1, NB):
        for j in range(i):
            out[:, :, i * P : (i + 1) * P, j * P : (j + 1) * P] = out[
                :, :, j * P : (j + 1) * P, i * P : (i + 1) * P
            ].transpose(0, 1, 3, 2)
    idx = np.arange(N)
    out[:, :, idx, idx] = 0.0
    return out


def kernel(async_fea: np.ndarray, coord: np.ndarray) -> np.ndarray:
    from concourse import bass_utils

    nc = _get_module()
    in_maps = _prep_inputs(async_fea, coord)
    res = bass_utils.run_bass_kernel_spmd(nc, in_maps, core_ids=list(range(NCORES)))
    return _unpack(res)


def kernel_traced(async_fea: np.ndarray, coord: np.ndarray):
    """Like kernel() but with NTFF tracing; returns (output, exec_time_ns)."""
    from concourse import bass_utils

    nc = _get_module()
    in_maps = _prep_inputs(async_fea, coord)
    res = bass_utils.run_bass_kernel_spmd(
        nc, in_maps, core_ids=list(range(NCORES)), trace=True
    )
    return _unpack(res), res.exec_time_ns


# revision 7
# speedup vs baseline: 1.2344x; 1.2344x over previous
"""Trainium2 Bass kernel for the ASG adjacency problem (v3, packed-symmetric).

Computes, for batched inputs async_fea [B,N,D] and coord [B,N,2]:
    fn   = async_fea / ||async_fea||_row      (host, f64 -> bf16)
    cos  = fn @ fn^T                          (PE, bf16 x bf16 -> f32 PSUM)
    d2   = (xi-xj)^2 + (yi-yj)^2              (exact f32 in reference rounding
                                               order: sync path needs this;
                                               8 pairs sit within 1e-6 of 1.0)
    async_adj = cos * exp(-sqrt(d2))          (bf16 out; loose-tol path)
    sync_adj  = (d2 < 1.0)                    (uint8 out)
Outputs are symmetric: the device computes only the packed upper block
triangle ([P, SW] per batch, SW=1280) and the host mirrors the lower
blocks, zeroes the diagonal, and upcasts to f32.

Packed block offsets are permuted (_OFF = [0, 512, 1024, 896]) so each
block's span stays inside a single 2KB PSUM bank, letting all 4 cos
matmuls target one [P, SW] PSUM region and the asy multiply run as a
single 1280-wide DVE op.

Sharding: data-parallel over batch: 8 NeuronCores x 8 batches.

Engine split (measured: ACT 0.92 / DVE-tt 0.69 / Pool 2.2-2.4 ns per
128-lane elem-row; DVE tensor_scalar is 2.5x slower than tensor_tensor,
so all DVE ops use tensor_tensor with stride-0 [P,1] broadcasts):
 - ACT: x-squares blocks {0,1}, y-squares block {2} (Square scale=-1
   bias: exact), dist=sqrt(d2), e=exp(-dist) -- sqrt/exp pair-wide
 - DVE: x-subs blocks {2,3} + xmul, y-subs blocks {0,1,3} + ymul,
   syn=is_lt (pair-wide), asy=cos*e (one 1280-wide op per batch)
 - Pool: d2 = xsq + ysq (pair-wide, in-place into xsq)
ACT table thrash avoided by phase-grouping (Square -> Sqrt -> Exp) per
GRP=4 batches, chained with scheduling-only deps.
"""

from contextlib import ExitStack

import numpy as np

import concourse.bacc as bacc
import concourse.bass as bass
import concourse.tile as tile
from concourse import mybir
from concourse.tile_rust import add_dep_helper

P = 128          # partitions
N = 512          # nodes per batch
D = 128          # feature dim
B = 64           # total batches
NCORES = 8
BPC = B // NCORES   # batches per core
NPAIR = BPC // 2    # batch pairs per core
NB = N // P         # 4 row blocks
SW = NB * (NB + 1) // 2 * P  # packed upper-tri width: 1280
GRP = 4             # batches per ACT phase group
F32 = mybir.dt.float32
BF16 = mybir.dt.bfloat16
U8 = mybir.dt.uint8

_AF = mybir.ActivationFunctionType
_OP = mybir.AluOpType

# packed column offset of row-block i inside the [P, SW] tiles; permuted so
# every block [off, off+W) stays inside one 2KB PSUM bank
_OFF = [0, 512, 1024, 896]
_W = [512, 384, 256, 128]


def _bc(ap, w):
    """[P,1] AP -> [P,w] stride-0 free-dim broadcast."""
    return bass.AP(ap.tensor, ap.offset, [[ap.ap[0][0], P], [0, w]])


def _build_module() -> bass.Bass:
    nc = bacc.Bacc(
        "TRN2", target_bir_lowering=False, debug=False, num_devices=NCORES
    )
    # fnT2[pr, d, q*N + j] = fn[2*pr+q, j, d]  (host-normalized, bf16)
    fnT2 = nc.declare_dram_parameter("fnT2", [NPAIR, P, 2 * N], BF16, isOutput=False)
    # cpair[pr, 0, :] = [x_{b0}(N) | y_{b0}(N) | x_{b1}(N) | y_{b1}(N)]
    cpair = nc.declare_dram_parameter("cpair", [NPAIR, 1, 4 * N], F32, isOutput=False)
    # scal[p, b*8 + c*4 + k] = coord[b, k*128+p, c]
    scal = nc.declare_dram_parameter("scal", [P, BPC * 8], F32, isOutput=False)
    oasy = nc.declare_dram_parameter("oasy", [NPAIR, P, 2 * SW], BF16, isOutput=True)
    osyn = nc.declare_dram_parameter("osyn", [NPAIR, P, 2 * SW], F32, isOutput=True)

    with tile.TileContext(nc) as tc, ExitStack() as ctx:
        _build_kernel(ctx, tc, fnT2, cpair, scal, oasy, osyn)
    nc.finalize()
    return nc


def _build_kernel(ctx, tc, fnT2, cpair, scal, oasy, osyn):
    nc = tc.nc
    prev_phase = []
    cur_acts = []

    def act(*args, **kwargs):
        """ScalarE activation ordered after every op of the previous
        *phase* so activation-table-load locality holds."""
        inst = nc.scalar.activation(*args, **kwargs)
        for p in prev_phase:
            add_dep_helper(inst.ins, p.ins, False, "act phase order")
        cur_acts.append(inst)
        return inst

    def act_phase():
        if cur_acts:
            prev_phase[:] = cur_acts
            cur_acts.clear()

    const_pool = ctx.enter_context(tc.tile_pool(name="const", bufs=1))
    cb_pool = ctx.enter_context(tc.tile_pool(name="cb", bufs=3))
    fn_pool = ctx.enter_context(tc.tile_pool(name="fn", bufs=3))
    sq_pool = ctx.enter_context(tc.tile_pool(name="sq", bufs=3))
    ty_pool = ctx.enter_context(tc.tile_pool(name="ty", bufs=2))
    de_pool = ctx.enter_context(tc.tile_pool(name="de", bufs=3))
    out_pool = ctx.enter_context(tc.tile_pool(name="outp", bufs=2))
    ps_pool = ctx.enter_context(tc.tile_pool(name="ps", bufs=2, space="PSUM"))

    scal_sb = const_pool.tile([P, BPC * 8], F32)
    nc.sync.dma_start(out=scal_sb[:], in_=scal[:])
    one_c = const_pool.tile([P, 1], F32)
    nc.vector.memset(one_c[:], 1.0)

    def sx(b, k):
        return scal_sb[:, b * 8 + k : b * 8 + k + 1]

    def sy(b, k):
        return scal_sb[:, b * 8 + 4 + k : b * 8 + 4 + k + 1]

    for g in range(BPC // GRP):
        prs = range(g * (GRP // 2), (g + 1) * (GRP // 2))

        cbs = {}
        fns = {}
        for pr in prs:
            cb = cb_pool.tile([P, 4 * N], F32, name="cb")
            a = cpair[pr, 0:1, :]
            nc.sync.dma_start(
                out=cb[:],
                in_=bass.AP(a.tensor, a.offset, [[1, 1], [0, P], [1, 4 * N]]),
            )
            cbs[pr] = cb
            fnt = fn_pool.tile([P, 2 * N], BF16, name="fnt")
            nc.sync.dma_start(out=fnt[:], in_=fnT2[pr])
            fns[pr] = fnt

        # ---- Phase A (Square table): squares, d2, syn -------------------
        d2s = {}
        syn2s = {}
        for pr in prs:
            cb = cbs[pr]
            xsq2 = sq_pool.tile([P, 2 * SW], F32, name="xsq2")
            ysq2 = ty_pool.tile([P, 2 * SW], F32, name="ysq2")
            for q in (0, 1):
                b = 2 * pr + q
                xof = q * 2 * N        # x row of batch q inside cb
                yof = q * 2 * N + N    # y row of batch q inside cb
                o = q * SW             # packed offset of batch q

                for i in range(NB):
                    W, c0, off = _W[i], i * P, _OFF[i]
                    if i < 2:
                        # ACT x-squares blocks 0,1 (exact)
                        act(out=xsq2[:, o + off : o + off + W],
                            in_=cb[:, xof + c0 : xof + N],
                            func=_AF.Square, bias=sx(b, i), scale=-1.0)
                    else:
                        # DVE x-subs blocks 2,3
                        nc.vector.tensor_tensor(
                            out=xsq2[:, o + off : o + off + W],
                            in0=cb[:, xof + c0 : xof + N],
                            in1=_bc(sx(b, i), W), op=_OP.subtract,
                        )
                    if i == 0:
                        # DVE y-sub block 0
                        nc.vector.tensor_tensor(
                            out=ysq2[:, o + off : o + off + W],
                            in0=cb[:, yof + c0 : yof + N],
                            in1=_bc(sy(b, i), W), op=_OP.subtract,
                        )
                    else:
                        # ACT y-squares blocks 1,2,3 (exact)
                        act(out=ysq2[:, o + off : o + off + W],
                            in_=cb[:, yof + c0 : yof + N],
                            func=_AF.Square, bias=sy(b, i), scale=-1.0)
                # xmul over blocks 3,2 = [896:1280); ymul over block 0 = [0:512)
                nc.vector.tensor_mul(
                    xsq2[:, o + 896 : o + SW],
                    xsq2[:, o + 896 : o + SW], xsq2[:, o + 896 : o + SW],
                )
                nc.vector.tensor_mul(
                    ysq2[:, o : o + 512],
                    ysq2[:, o : o + 512], ysq2[:, o : o + 512],
                )
                # d2 = xsq + ysq in place (exact f32 add), split Pool/DVE
                nc.gpsimd.tensor_add(
                    xsq2[:, o : o + 512], xsq2[:, o : o + 512],
                    ysq2[:, o : o + 512],
                )
                nc.vector.tensor_add(
                    xsq2[:, o + 512 : o + SW], xsq2[:, o + 512 : o + SW],
                    ysq2[:, o + 512 : o + SW],
                )
            d2s[pr] = xsq2
            syn2 = out_pool.tile([P, 2 * SW], F32, name="syn2")
            nc.vector.tensor_tensor(
                out=syn2[:], in0=xsq2[:], in1=_bc(one_c[:, 0:1], 2 * SW),
                op=_OP.is_lt,
            )
            syn2s[pr] = syn2
            nc.sync.dma_start(out=osyn[pr], in_=syn2[:])

        act_phase()
        # ---- Phase B (Sqrt): dist ---------------------------------------
        dists = {}
        for pr in prs:
            dist2 = de_pool.tile([P, 2 * SW], BF16, name="dist2")
            act(out=dist2[:], in_=d2s[pr][:], func=_AF.Sqrt)
            dists[pr] = dist2

        act_phase()
        # ---- Phase C (Exp): e, cos, asy ---------------------------------
        for pr in prs:
            fnt = fns[pr]
            e2 = de_pool.tile([P, 2 * SW], BF16, name="e2")
            act(out=e2[:], in_=dists[pr][:], func=_AF.Exp, scale=-1.0)
            asy2 = out_pool.tile([P, 2 * SW], BF16, name="asy2")
            for q in (0, 1):
                b = 2 * pr + q
                # [P, 1536] = 3 PSUM banks so block spans stay bank-local
                ps = ps_pool.tile([P, 1536], F32, name="ps")
                for i in range(NB):
                    W, c0, off = _W[i], i * P, _OFF[i]
                    nc.tensor.matmul(
                        ps[:, off : off + W],
                        lhsT=fnt[:, q * N + c0 : q * N + c0 + P],
                        rhs=fnt[:, q * N + c0 : (q + 1) * N],
                        start=True, stop=True,
                    )
                nc.vector.tensor_mul(
                    asy2[:, q * SW : (q + 1) * SW], ps[:, :SW],
                    e2[:, q * SW : (q + 1) * SW],
                )
            nc.sync.dma_start(out=oasy[pr], in_=asy2[:])
        act_phase()


_NC_CACHE = None


def _get_module():
    global _NC_CACHE
    if _NC_CACHE is None:
        _NC_CACHE = _build_module()
    return _NC_CACHE


def _prep_inputs(async_fea: np.ndarray, coord: np.ndarray):
    import ml_dtypes

    fea = np.asarray(async_fea, dtype=np.float32)
    crd = np.asarray(coord, dtype=np.float32)
    # host-side row normalization (f64 for accuracy; cos path is loose-tol)
    nrm = np.maximum(
        np.sqrt((fea.astype(np.float64) ** 2).sum(-1, keepdims=True)), 1e-8
    )
    fn = (fea.astype(np.float64) / nrm).astype(ml_dtypes.bfloat16)

    in_maps = []
    for c in range(NCORES):
        sl = slice(c * BPC, (c + 1) * BPC)
        fn_c = fn[sl]                      # [BPC, N, D]
        # fnT2[pr, d, q*N + j] = fn[2pr+q, j, d]
        fnT2 = np.ascontiguousarray(
            fn_c.reshape(NPAIR, 2, N, D).transpose(0, 3, 1, 2).reshape(
                NPAIR, D, 2 * N
            )
        )
        cT = crd[sl].transpose(0, 2, 1)    # [BPC, 2, N]
        # [pr, q, c, N] row-major -> [x_b0 | y_b0 | x_b1 | y_b1]
        cpair = np.ascontiguousarray(cT.reshape(NPAIR, 1, 4 * N))
        # scal[p, b*8 + c*4 + k] = coord[b, k*128+p, c]
        sc = np.ascontiguousarray(
            cT.reshape(BPC, 2, NB, P).transpose(3, 0, 1, 2).reshape(P, BPC * 8)
        )
        in_maps.append({"fnT2": fnT2, "cpair": cpair, "scal": sc})
    return in_maps


def _unpack(res) -> np.ndarray:
    """Packed per-core [NPAIR, P, 2*SW] outputs -> full [2, B, N, N] f32."""
    asy = np.concatenate(
        [np.asarray(res.results[c]["oasy"]) for c in range(NCORES)], axis=0
    )
    syn = np.concatenate(
        [np.asarray(res.results[c]["osyn"]) for c in range(NCORES)], axis=0
    )
    asy = asy.reshape(B // 2, P, 2, SW).transpose(0, 2, 1, 3).reshape(B, P, SW)
    syn = syn.reshape(B // 2, P, 2, SW).transpose(0, 2, 1, 3).reshape(B, P, SW)

    out = np.empty((2, B, N, N), dtype=np.float32)
    for i in range(NB):
        W, c0, off = _W[i], i * P, _OFF[i]
        out[0, :, c0 : c0 + P, c0:] = asy[:, :, off : off + W]
        out[1, :, c0 : c0 + P, c0:] = syn[:, :, off : off + W]
    # mirror lower blocks from the (computed) upper blocks
    for i in range(1, NB):
        for j in range(i):
            out[:, :, i * P : (i + 1) * P, j * P : (j + 1) * P] = out[
                :, :, j * P : (j + 1) * P, i * P : (i + 1) * P
            ].transpose(0, 1, 3, 2)
    idx = np.arange(N)
    out[:, :, idx, idx] = 0.0
    return out


def kernel(async_fea: np.ndarray, coord: np.ndarray) -> np.ndarray:
    from concourse import bass_utils

    nc = _get_module()
    in_maps = _prep_inputs(async_fea, coord)
    res = bass_utils.run_bass_kernel_spmd(nc, in_maps, core_ids=list(range(NCORES)))
    return _unpack(res)


def kernel_traced(async_fea: np.ndarray, coord: np.ndarray):
    """Like kernel() but with NTFF tracing; returns (output, exec_time_ns)."""
    from concourse import bass_utils

    nc = _get_module()
    in_maps = _prep_inputs(async_fea, coord)
    res = bass_utils.run_bass_kernel_spmd(
        nc, in_maps, core_ids=list(range(NCORES)), trace=True
    )
    return _unpack(res), res.exec_time_ns
